# revision 32
# baseline (speedup 1.0000x reference)
"""Multi-head attention (16 heads, DM=1024, DK=DV=64, S=2048, B=2, causal)
tensor-parallel over heads on 8 NeuronCores (2 heads per core).

Host-side marshalling:
  - Activations pre-transposed to XT[B, DM, S] bf16 so device matmuls
    contract over DM on the partition dimension with natural layouts.
  - Per-core weights: WQ/WK/WV head pair stacked on columns -> [DM, 128]
    bf16; WO rows for the head pair -> [128, DM] bf16.
  - Each core computes its 2 heads end-to-end plus the WO partial
    projection; host sums the 8 partial outputs.

Device pipeline, fused per (batch b, query chunk c of 512):
  - proj: qT/kT [128(2h x dk), S] = W.T @ XT chunks; v [s 128, 130] per
    s_k tile = [v_h0(64) | 1 | v_h1(64) | 1] (ones cols give row-sums
    for free in the PV matmul's 65th output row).
  - attention rounds, one s_k tile each, st [128,1024] = [h0|h1]
    scores: the two score matmuls are issued adjacently -> the PE runs
    them concurrently in different row groups (K=64 each, 2x
    throughput); one exp on ScalarE per round covers both heads;
    causal mask multiply (DVE) only on the aligned 128x128 diagonal
    block; PV accumulates [65, 512] per head over s_k tiles in
    DESCENDING order with causally-trimmed N (ragged accumulation via
    per-element has_written; CoreSim can't model it -> trim=False).
  - normalize: rowsum -> 1/r via Ln then Exp(-x) on ScalarE (cheap,
    co-resident table set); broadcast to 64 partitions via a K=1
    matmul; fused multiply-evacuate to oT bf16.
  - WO: y chunk = oT.T @ WO (bf16, full rate), evacuate bf16, DMA out.
"""

import numpy as np

S, B, DM, DK, DV, H = 2048, 2, 1024, 64, 64, 16
NCORES = 8
HEADS_PER_CORE = H // NCORES  # 2
SCALE = 1.0 / np.sqrt(DK)  # 1/8

_CACHE = {}


def build_nc(split_waits=True, trim=True):
    # trim=True uses ragged causally-trimmed PV accumulation (descending
    # s_k order, per-element has_written semantics). Real HW supports it;
    # CoreSim's accumulation model does not, so sim runs use trim=False.
    import concourse.bass as bass
    import concourse.tile as tile
    from concourse import mybir

    f32 = mybir.dt.float32
    bf16 = mybir.dt.bfloat16
    Exp = mybir.ActivationFunctionType.Exp
    Ln = mybir.ActivationFunctionType.Ln
    mult = mybir.AluOpType.mult
    nc = bass.Bass()

    xtq = nc.dram_tensor("xtq", [B, DM, S], bf16, kind="ExternalInput")
    xtk = nc.dram_tensor("xtk", [B, DM, S], bf16, kind="ExternalInput")
    xtv = nc.dram_tensor("xtv", [B, DM, S], bf16, kind="ExternalInput")
    wq = nc.dram_tensor("wq", [DM, 128], bf16, kind="ExternalInput")
    wk = nc.dram_tensor("wk", [DM, 128], bf16, kind="ExternalInput")
    wv = nc.dram_tensor("wv", [DM, 128], bf16, kind="ExternalInput")
    wo = nc.dram_tensor("wo", [128, DM], bf16, kind="ExternalInput")
    masks = nc.dram_tensor("masks", [4, 128, 512], bf16, kind="ExternalInput")
    ident = nc.dram_tensor("ident", [128, 128], bf16, kind="ExternalInput")
    y = nc.dram_tensor("y", [S, B, DM], bf16, kind="ExternalOutput")

    NJ = DM // 128  # 8 contraction chunks
    NC_Q = S // 512  # 4 s_q chunks per batch
    NT = S // 128  # 16 s_k tiles per batch
    VW = 130  # per-s_k-tile v storage: [v_h0(64) | 1 | v_h1(64) | 1]

    with tile.TileContext(nc) as tc:
        with (
            tc.tile_pool(name="const", bufs=1) as const,
            tc.tile_pool(name="xt", bufs=3) as xtp,
            tc.tile_pool(name="qkv", bufs=2) as qkvp,
            tc.tile_pool(name="pt", bufs=4) as ptp,
            tc.tile_pool(name="osb", bufs=2) as osbp,
            tc.tile_pool(name="sm", bufs=2) as smp,
            tc.tile_pool(name="ysbp", bufs=5) as ysbp,
            tc.tile_pool(name="psS", bufs=2, space="PSUM") as psS,
            tc.tile_pool(name="psO", bufs=1, space="PSUM") as psO,
            tc.tile_pool(name="psM", bufs=2, space="PSUM") as psM,
        ):
            # ---- constants ----
            wq_sb = const.tile([128, DM], bf16)
            wk_sb = const.tile([128, DM], bf16)
            wv_sb = const.tile([128, DM], bf16)
            wo_sb = const.tile([128, DM], bf16)
            masks_sb = const.tile([128, 4 * 512], bf16)
            ones_sb = const.tile([1, 512], bf16)
            id_sb = const.tile([128, 128], bf16)
            nc.sync.dma_start(out=wq_sb.rearrange("p (j c) -> p j c", c=128),
                              in_=wq[:, :].rearrange("(j p) c -> p j c", p=128))
            nc.sync.dma_start(out=wk_sb.rearrange("p (j c) -> p j c", c=128),
                              in_=wk[:, :].rearrange("(j p) c -> p j c", p=128))
            nc.sync.dma_start(out=wv_sb.rearrange("p (j c) -> p j c", c=128),
                              in_=wv[:, :].rearrange("(j p) c -> p j c", p=128))
            nc.sync.dma_start(out=wo_sb[:], in_=wo[:, :])
            nc.sync.dma_start(out=masks_sb.rearrange("p (d q) -> p d q", q=512),
                              in_=masks[:, :, :].rearrange("d p q -> p d q"))
            nc.sync.dma_start(out=id_sb[:], in_=ident[:, :])
            nc.vector.memset(ones_sb[:], 1.0)

            wo_queue = []
            pending_norm = []

            def _emit_norm():
                # rps matmuls + fused multiply-evacuate for the previous
                # chunk. Emitted AFTER the next chunk's projections so the
                # rcp dependency chain never heads-of-line-blocks the PE.
                nb, nc_, not0, not1, noT, nrcp = pending_norm.pop()
                rps_ps = psM.tile([128, 512], f32, tag="mm")
                for h in (0, 1):  # both heads -> one bank, concurrent MMs
                    nc.tensor.matmul(rps_ps[h * 64:(h + 1) * 64, :],
                                     ones_sb[0:1, 0:64],
                                     nrcp[0:1, h * 512:(h + 1) * 512],
                                     start=True, stop=True)
                for h, otx in ((0, not0), (1, not1)):
                    rps_sb = smp.tile([64, 512], bf16, tag=f"rps{h}")
                    nc.vector.tensor_copy(rps_sb[:],
                                          rps_ps[h * 64:(h + 1) * 64, :])
                    nc.vector.tensor_tensor(
                        out=noT[h * 64:h * 64 + 64, nc_ * 512:(nc_ + 1) * 512],
                        in0=otx[0:64, :], in1=rps_sb[:], op=mult)
                for t in range(4 * nc_, 4 * nc_ + 4):
                    wo_queue.append((nb, noT, t))

            def _emit_wo(item, tail=False):
                wb, woT, wt = item
                ysb = ysbp.tile([128, 1024], bf16, tag="ysb")
                for wm in range(2):
                    yps = psM.tile([128, 512], f32, tag="mm")
                    nc.tensor.matmul(yps[:],
                                     woT[:, wt * 128:(wt + 1) * 128],
                                     wo_sb[:, wm * 512:(wm + 1) * 512],
                                     start=True, stop=True)
                    if tail and wm == 1:  # spread drain over ACT too
                        nc.scalar.copy(ysb[:, wm * 512:(wm + 1) * 512], yps[:])
                    else:
                        nc.vector.tensor_copy(
                            ysb[:, wm * 512:(wm + 1) * 512], yps[:])
                nc.sync.dma_start(
                    out=y[wt * 128:(wt + 1) * 128, wb, :],
                    in_=ysb[:])

            def _emit_dmas(db, dc):
                xq = xtp.tile([128, NJ * 512], bf16, tag="xq")
                xk = xtp.tile([128, NJ * 512], bf16, tag="xk")
                xv = xtp.tile([128, NJ * 512], bf16, tag="xv")
                for xt_sb, xt_dram in ((xq, xtq), (xk, xtk), (xv, xtv)):
                    ov = xt_sb.rearrange("p (j s) -> p j s", s=512)
                    iv = xt_dram[db].rearrange("(j p) s -> p j s", p=128)
                    for g in range(4):  # 4 DMAs/input -> 12 queues busy
                        nc.sync.dma_start(
                            out=ov[:, 2 * g:2 * g + 2, :],
                            in_=iv[:, 2 * g:2 * g + 2,
                                   dc * 512:(dc + 1) * 512])
                return xq, xk, xv

            # HAM warm-up: ~9us of dependency-free dummy matmuls so the PE
            # clock is at 8/8 when the first projections arrive (covers the
            # initial input-DMA window; PE would be idle regardless).
            warm_ps = psO.tile([64, 512], f32, tag="ot0")
            for _ in range(24):
                nc.tensor.matmul(warm_ps[:], ones_sb[0:1, 0:64], ones_sb[:],
                                 start=True, stop=True)

            for b in range(B):
                qT = qkvp.tile([128, S], bf16, tag="qT")
                kT = qkvp.tile([128, S], bf16, tag="kT")
                v_sb = qkvp.tile([128, NT * VW], bf16, tag="v")
                oT = osbp.tile([128, S], bf16, tag="oT")
                vv = v_sb.rearrange("p (t w) -> p t w", w=VW)
                nc.vector.memset(vv[:, :, 64:65], 1.0)
                nc.vector.memset(vv[:, :, 129:130], 1.0)

                for c in range(NC_Q):
                    # ---------- input DMA for this chunk ----------
                    xq, xk, xv = _emit_dmas(b, c)

                    # ---------- projections ----------
                    ps_q = psM.tile([128, 512], f32, tag="mm")
                    for j in range(NJ):
                        nc.tensor.matmul(ps_q[:], wq_sb[:, j * 128:(j + 1) * 128],
                                         xq[:, j * 512:(j + 1) * 512],
                                         start=(j == 0), stop=(j == NJ - 1))
                    nc.vector.tensor_copy(qT[:, c * 512:(c + 1) * 512], ps_q[:])
                    ps_k = psM.tile([128, 512], f32, tag="mm")
                    for j in range(NJ):
                        nc.tensor.matmul(ps_k[:], wk_sb[:, j * 128:(j + 1) * 128],
                                         xk[:, j * 512:(j + 1) * 512],
                                         start=(j == 0), stop=(j == NJ - 1))
                    nc.vector.tensor_copy(kT[:, c * 512:(c + 1) * 512], ps_k[:])
                    # v: project transposed (N=512, full PE rate) then
                    # PE-transpose the four 128x128 s-tiles into [s, hd]
                    ps_vt = psM.tile([128, 512], f32, tag="mm")
                    for j in range(NJ):
                        nc.tensor.matmul(ps_vt[:], wv_sb[:, j * 128:(j + 1) * 128],
                                         xv[:, j * 512:(j + 1) * 512],
                                         start=(j == 0), stop=(j == NJ - 1))
                    vt_sb = smp.tile([128, 512], bf16, tag="vt")
                    nc.vector.tensor_copy(vt_sb[:], ps_vt[:])
                    ps_v = psM.tile([128, 512], bf16, tag="mm")
                    for u in range(4):
                        nc.tensor.transpose(ps_v[:, u * 128:(u + 1) * 128],
                                            vt_sb[:, u * 128:(u + 1) * 128],
                                            id_sb[:])
                    nc.vector.tensor_copy(
                        vv[:, 4 * c:4 * c + 4, 0:130]
                        .rearrange("p u (h w) -> p u h w", h=2)[:, :, :, 0:64],
                        ps_v[:].rearrange("p (u h w) -> p u h w", u=4, h=2))

                    # previous chunk's normalize (rcp is ready by now —
                    # its Ln/Exp ran on ACT during our projections)
                    if pending_norm:
                        _emit_norm()

                    # ---------- attention rounds (one s_k tile each) ----------
                    # st [128, 1024] = [h0 scores | h1 scores] for tile t.
                    # bufs=2 double-buffers: scores(t+1) run during exp(t).
                    n_t = 4 * c + 4
                    ot0 = psO.tile([65, 512], f32, tag="ot0")
                    ot1 = psO.tile([65, 512], f32, tag="ot1")
                    wo_work = wo_queue
                    wo_queue = []
                    wo_per_round = (len(wo_work) + n_t - 1) // n_t if wo_work else 0
                    # s_k tiles DESCENDING: the widest PV matmul goes first
                    # (start=True clears the whole bank; later narrower tiles
                    # overwrite-where-unwritten / accumulate-where-written),
                    # making causally-trimmed ragged accumulation legal. The
                    # mask then only covers the aligned 128x128 block.
                    for t in range(n_t - 1, -1, -1):
                        st = psS.tile([128, 1024], f32, tag="st")
                        for h in (0, 1):
                            hp = h * 64
                            nc.tensor.matmul(
                                st[:, h * 512:(h + 1) * 512],
                                kT[hp:hp + 64, t * 128:(t + 1) * 128],
                                qT[hp:hp + 64, c * 512:(c + 1) * 512],
                                start=True, stop=True)
                        pt = ptp.tile([128, 1024], bf16, tag="pt")
                        nc.scalar.activation(pt[:], st[:], Exp, scale=float(SCALE))
                        off = max(0, (t - 4 * c) * 128) if trim else 0
                        if t >= 4 * c:  # diagonal tile: mask aligned block
                            dd = t - 4 * c
                            mw = 128 if trim else (dd + 1) * 128
                            for h in (0, 1):
                                nc.vector.tensor_tensor(
                                    out=pt[:, h * 512 + off:h * 512 + off + mw],
                                    in0=pt[:, h * 512 + off:h * 512 + off + mw],
                                    in1=masks_sb[:, dd * 512 + off:
                                                 dd * 512 + off + mw],
                                    op=mult)
                        for h, otx in ((0, ot0), (1, ot1)):
                            nc.tensor.matmul(
                                otx[0:65, off:512],
                                v_sb[:, t * VW + h * 65:t * VW + h * 65 + 65],
                                pt[:, h * 512 + off:(h + 1) * 512],
                                start=(t == n_t - 1), stop=(t == 0))
                        for _ in range(wo_per_round):
                            if wo_work:
                                _emit_wo(wo_work.pop(0))
                    while wo_work:
                        _emit_wo(wo_work.pop(0))

                    # ---------- rowsum -> 1/r (ACT only; rest deferred) ----------
                    lnr = smp.tile([1, 1024], f32, tag="lnr")
                    nc.scalar.activation(lnr[0:1, 0:512], ot0[64:65, :], Ln)
                    nc.scalar.activation(lnr[0:1, 512:1024], ot1[64:65, :], Ln)
                    rcp = smp.tile([1, 1024], bf16, tag="rcp")
                    nc.scalar.activation(rcp[:], lnr[:], Exp, scale=-1.0)
                    pending_norm.append((b, c, ot0, ot1, oT, rcp))
            _emit_norm()  # the very last chunk's normalize + WO drain
            while wo_queue:
                _emit_wo(wo_queue.pop(0), tail=True)
    if split_waits:
        _split_waits(nc, mybir)
    return nc


def _split_waits(nc, mybir):
    """This walrus build encodes at most ONE sync wait per instruction.
    Instructions with a single wait keep it inline (free); only multi-
    wait instructions get the extra waits hoisted onto same-engine NoOps
    issued immediately before — semantically identical: the sequencer
    blocks at the NoOp instead."""
    ctr = [0]
    for fn in nc.m.functions:
        for blk in fn.blocks:
            new_insts = []
            for ins in blk.instructions:
                si = getattr(ins, "sync_info", None)
                waits = list(si.on_wait) if si is not None and si.on_wait else []
                if len(waits) > 1:
                    for w in waits[:-1]:
                        ctr[0] += 1
                        nop = mybir.InstNoOp(name=f"WSPLIT-{ctr[0]}", ins=[], outs=[])
                        nop.engine = ins.engine
                        nop.sync_info = mybir.SyncInfo(on_wait=[w], on_update=[])
                        new_insts.append(nop)
                    ins.sync_info = mybir.SyncInfo(
                        on_wait=[waits[-1]], on_update=list(si.on_update or []))
                new_insts.append(ins)
            blk.instructions = new_insts


def _marshal(Q, K, V, WQ, WK, WV, WO):
    Q = np.asarray(Q, dtype=np.float32)
    K = np.asarray(K, dtype=np.float32)
    V = np.asarray(V, dtype=np.float32)
    WQ = np.asarray(WQ, dtype=np.float32)
    WK = np.asarray(WK, dtype=np.float32)
    WV = np.asarray(WV, dtype=np.float32)
    WO = np.asarray(WO, dtype=np.float32)

    import ml_dtypes
    bf = ml_dtypes.bfloat16
    xtq = np.ascontiguousarray(Q.transpose(1, 2, 0)).astype(bf)  # [B, DM, S]
    xtk = np.ascontiguousarray(K.transpose(1, 2, 0)).astype(bf)
    xtv = np.ascontiguousarray(V.transpose(1, 2, 0)).astype(bf)

    masks = np.zeros((4, 128, 512), dtype=bf)
    kk = np.arange(128)[:, None]
    qq = np.arange(512)[None, :]
    for d in range(4):
        masks[d] = (d * 128 + kk <= qq).astype(bf)

    in_maps = []
    for core in range(NCORES):
        h0 = core * HEADS_PER_CORE
        wql = np.ascontiguousarray(np.concatenate([WQ[h0], WQ[h0 + 1]], axis=1)).astype(bf)
        wkl = np.ascontiguousarray(np.concatenate([WK[h0], WK[h0 + 1]], axis=1)).astype(bf)
        wvl = np.ascontiguousarray(np.concatenate([WV[h0], WV[h0 + 1]], axis=1)).astype(bf)
        wol = np.ascontiguousarray(WO[h0 * DV:(h0 + 2) * DV, :]).astype(bf)
        in_maps.append({
            "xtq": xtq, "xtk": xtk, "xtv": xtv,
            "wq": wql, "wk": wkl, "wv": wvl, "wo": wol,
            "masks": masks, "ident": np.eye(128, dtype=bf),
        })
    return in_maps


LAST_RESULTS = None


def kernel(Q, K, V, WQ, WK, WV, WO):
    global LAST_RESULTS
    from concourse.bass_utils import run_bass_kernel_spmd

    if "nc" not in _CACHE:
        _CACHE["nc"] = build_nc()
    nc = _CACHE["nc"]

    in_maps = _marshal(Q, K, V, WQ, WK, WV, WO)
    res = run_bass_kernel_spmd(nc, in_maps, core_ids=list(range(NCORES)))
    LAST_RESULTS = res
    out = np.zeros((S, B, DM), dtype=np.float32)
    for r in res.results:
        out += np.asarray(r["y"]).astype(np.float32)
    return out


# revision 34
# speedup vs baseline: 1.0517x; 1.0517x over previous
"""Multi-head attention (16 heads, DM=1024, DK=DV=64, S=2048, B=2, causal)
tensor-parallel over heads on 8 NeuronCores (2 heads per core).

Host-side marshalling:
  - Activations pre-transposed to XT[B, DM, S] bf16 so device matmuls
    contract over DM on the partition dimension with natural layouts.
  - Per-core weights: WQ/WK/WV head pair stacked on columns -> [DM, 128]
    bf16; WO rows for the head pair -> [128, DM] bf16.
  - Each core computes its 2 heads end-to-end plus the WO partial
    projection; host sums the 8 partial outputs.

Device pipeline, fused per (batch b, query chunk c of 512):
  - proj: qT/kT [128(2h x dk), S] = W.T @ XT chunks; v [s 128, 130] per
    s_k tile = [v_h0(64) | 1 | v_h1(64) | 1] (ones cols give row-sums
    for free in the PV matmul's 65th output row).
  - attention rounds, one s_k tile each, st [128,1024] = [h0|h1]
    scores: the two score matmuls are issued adjacently -> the PE runs
    them concurrently in different row groups (K=64 each, 2x
    throughput); one exp on ScalarE per round covers both heads;
    causal mask multiply (DVE) only on the aligned 128x128 diagonal
    block; PV accumulates [65, 512] per head over s_k tiles in
    DESCENDING order with causally-trimmed N (ragged accumulation via
    per-element has_written; CoreSim can't model it -> trim=False).
  - normalize: rowsum -> 1/r via Ln then Exp(-x) on ScalarE (cheap,
    co-resident table set); broadcast to 64 partitions via a K=1
    matmul; fused multiply-evacuate to oT bf16.
  - WO: y chunk = oT.T @ WO (bf16, full rate), evacuate bf16, DMA out.
"""

import numpy as np

S, B, DM, DK, DV, H = 2048, 2, 1024, 64, 64, 16
NCORES = 8
HEADS_PER_CORE = H // NCORES  # 2
SCALE = 1.0 / np.sqrt(DK)  # 1/8

_CACHE = {}


def build_nc(split_waits=True, trim=True):
    # trim=True uses ragged causally-trimmed PV accumulation (descending
    # s_k order, per-element has_written semantics). Real HW supports it;
    # CoreSim's accumulation model does not, so sim runs use trim=False.
    import concourse.bass as bass
    import concourse.tile as tile
    from concourse import mybir

    f32 = mybir.dt.float32
    bf16 = mybir.dt.bfloat16
    Exp = mybir.ActivationFunctionType.Exp
    Ln = mybir.ActivationFunctionType.Ln
    mult = mybir.AluOpType.mult
    nc = bass.Bass()

    xtq = nc.dram_tensor("xtq", [B, DM, S], bf16, kind="ExternalInput")
    xtk = nc.dram_tensor("xtk", [B, DM, S], bf16, kind="ExternalInput")
    xtv = nc.dram_tensor("xtv", [B, DM, S], bf16, kind="ExternalInput")
    wq = nc.dram_tensor("wq", [DM, 128], bf16, kind="ExternalInput")
    wk = nc.dram_tensor("wk", [DM, 128], bf16, kind="ExternalInput")
    wv = nc.dram_tensor("wv", [DM, 128], bf16, kind="ExternalInput")
    wo = nc.dram_tensor("wo", [128, DM], bf16, kind="ExternalInput")
    masks = nc.dram_tensor("masks", [4, 128, 512], bf16, kind="ExternalInput")
    y = nc.dram_tensor("y", [S, B, DM], bf16, kind="ExternalOutput")

    NJ = DM // 128  # 8 contraction chunks
    NC_Q = S // 512  # 4 s_q chunks per batch
    NT = S // 128  # 16 s_k tiles per batch
    VW = 130  # per-s_k-tile v storage: [v_h0(64) | 1 | v_h1(64) | 1]

    with tile.TileContext(nc) as tc:
        with (
            tc.tile_pool(name="const", bufs=1) as const,
            tc.tile_pool(name="xt", bufs=3) as xtp,
            tc.tile_pool(name="qkv", bufs=2) as qkvp,
            tc.tile_pool(name="pt", bufs=4) as ptp,
            tc.tile_pool(name="osb", bufs=2) as osbp,
            tc.tile_pool(name="sm", bufs=2) as smp,
            tc.tile_pool(name="ysbp", bufs=5) as ysbp,
            tc.tile_pool(name="psS", bufs=2, space="PSUM") as psS,
            tc.tile_pool(name="psO", bufs=1, space="PSUM") as psO,
            tc.tile_pool(name="psM", bufs=2, space="PSUM") as psM,
        ):
            # ---- constants ----
            wq_sb = const.tile([128, DM], bf16)
            wk_sb = const.tile([128, DM], bf16)
            wv_sb = const.tile([128, DM], bf16)
            wo_sb = const.tile([128, DM], bf16)
            masks_sb = const.tile([128, 4 * 512], bf16)
            ones_sb = const.tile([1, 512], bf16)
            nc.sync.dma_start(out=wq_sb.rearrange("p (j c) -> p j c", c=128),
                              in_=wq[:, :].rearrange("(j p) c -> p j c", p=128))
            nc.sync.dma_start(out=wk_sb.rearrange("p (j c) -> p j c", c=128),
                              in_=wk[:, :].rearrange("(j p) c -> p j c", p=128))
            nc.sync.dma_start(out=wv_sb.rearrange("p (j c) -> p j c", c=128),
                              in_=wv[:, :].rearrange("(j p) c -> p j c", p=128))
            nc.sync.dma_start(out=wo_sb[:], in_=wo[:, :])
            nc.sync.dma_start(out=masks_sb.rearrange("p (d q) -> p d q", q=512),
                              in_=masks[:, :, :].rearrange("d p q -> p d q"))
            nc.vector.memset(ones_sb[:], 1.0)

            wo_queue = []
            pending_norm = []

            def _emit_norm():
                # rps matmuls + fused multiply-evacuate for the previous
                # chunk. Emitted AFTER the next chunk's projections so the
                # rcp dependency chain never heads-of-line-blocks the PE.
                nb, nc_, not0, not1, noT, nrcp = pending_norm.pop()
                rps_ps = psM.tile([128, 512], f32, tag="mm")
                for h in (0, 1):  # both heads -> one bank, concurrent MMs
                    nc.tensor.matmul(rps_ps[h * 64:(h + 1) * 64, :],
                                     ones_sb[0:1, 0:64],
                                     nrcp[0:1, h * 512:(h + 1) * 512],
                                     start=True, stop=True)
                for h, otx in ((0, not0), (1, not1)):
                    rps_sb = smp.tile([64, 512], bf16, tag=f"rps{h}")
                    nc.vector.tensor_copy(rps_sb[:],
                                          rps_ps[h * 64:(h + 1) * 64, :])
                    nc.vector.tensor_tensor(
                        out=noT[h * 64:h * 64 + 64, nc_ * 512:(nc_ + 1) * 512],
                        in0=otx[0:64, :], in1=rps_sb[:], op=mult)
                for t in range(4 * nc_, 4 * nc_ + 4):
                    wo_queue.append((nb, noT, t))

            def _emit_wo(item, tail=False):
                wb, woT, wt = item
                ysb = ysbp.tile([128, 1024], bf16, tag="ysb")
                for wm in range(2):
                    yps = psM.tile([128, 512], f32, tag="mm")
                    nc.tensor.matmul(yps[:],
                                     woT[:, wt * 128:(wt + 1) * 128],
                                     wo_sb[:, wm * 512:(wm + 1) * 512],
                                     start=True, stop=True)
                    if tail and wm == 1:  # spread drain over ACT too
                        nc.scalar.copy(ysb[:, wm * 512:(wm + 1) * 512], yps[:])
                    else:
                        nc.vector.tensor_copy(
                            ysb[:, wm * 512:(wm + 1) * 512], yps[:])
                nc.sync.dma_start(
                    out=y[wt * 128:(wt + 1) * 128, wb, :],
                    in_=ysb[:])

            def _emit_dmas(db, dc):
                xq = xtp.tile([128, NJ * 512], bf16, tag="xq")
                xk = xtp.tile([128, NJ * 512], bf16, tag="xk")
                xv = xtp.tile([128, NJ * 512], bf16, tag="xv")
                for xt_sb, xt_dram in ((xq, xtq), (xk, xtk), (xv, xtv)):
                    ov = xt_sb.rearrange("p (j s) -> p j s", s=512)
                    iv = xt_dram[db].rearrange("(j p) s -> p j s", p=128)
                    for g in range(4):  # 4 DMAs/input -> 12 queues busy
                        nc.sync.dma_start(
                            out=ov[:, 2 * g:2 * g + 2, :],
                            in_=iv[:, 2 * g:2 * g + 2,
                                   dc * 512:(dc + 1) * 512])
                return xq, xk, xv

            # HAM warm-up: ~9us of dependency-free dummy matmuls so the PE
            # clock is at 8/8 when the first projections arrive (covers the
            # initial input-DMA window; PE would be idle regardless).
            warm_ps = psO.tile([64, 512], f32, tag="ot0")
            for _ in range(30):
                nc.tensor.matmul(warm_ps[:], ones_sb[0:1, 0:64], ones_sb[:],
                                 start=True, stop=True)

            for b in range(B):
                qT = qkvp.tile([128, S], bf16, tag="qT")
                kT = qkvp.tile([128, S], bf16, tag="kT")
                v_sb = qkvp.tile([128, NT * VW], bf16, tag="v")
                oT = osbp.tile([128, S], bf16, tag="oT")
                vv = v_sb.rearrange("p (t w) -> p t w", w=VW)
                nc.vector.memset(vv[:, :, 64:65], 1.0)
                nc.vector.memset(vv[:, :, 129:130], 1.0)

                for c in range(NC_Q):
                    # ---------- input DMA for this chunk ----------
                    xq, xk, xv = _emit_dmas(b, c)

                    # ---------- projections ----------
                    ps_q = psM.tile([128, 512], f32, tag="mm")
                    for j in range(NJ):
                        nc.tensor.matmul(ps_q[:], wq_sb[:, j * 128:(j + 1) * 128],
                                         xq[:, j * 512:(j + 1) * 512],
                                         start=(j == 0), stop=(j == NJ - 1))
                    nc.vector.tensor_copy(qT[:, c * 512:(c + 1) * 512], ps_q[:])
                    ps_k = psM.tile([128, 512], f32, tag="mm")
                    for j in range(NJ):
                        nc.tensor.matmul(ps_k[:], wk_sb[:, j * 128:(j + 1) * 128],
                                         xk[:, j * 512:(j + 1) * 512],
                                         start=(j == 0), stop=(j == NJ - 1))
                    nc.vector.tensor_copy(kT[:, c * 512:(c + 1) * 512], ps_k[:])
                    # v tiles: out [s 128, 128(2h x dv)], 4 per chunk in one bank
                    ps_v = psM.tile([128, 512], f32, tag="mm")
                    for u in range(4):
                        for j in range(NJ):
                            nc.tensor.matmul(
                                ps_v[:, u * 128:(u + 1) * 128],
                                xv[:, j * 512 + u * 128: j * 512 + (u + 1) * 128],
                                wv_sb[:, j * 128:(j + 1) * 128],
                                start=(j == 0), stop=(j == NJ - 1))
                    nc.vector.tensor_copy(
                        vv[:, 4 * c:4 * c + 4, 0:130]
                        .rearrange("p u (h w) -> p u h w", h=2)[:, :, :, 0:64],
                        ps_v[:].rearrange("p (u h w) -> p u h w", u=4, h=2))

                    # previous chunk's normalize (rcp is ready by now —
                    # its Ln/Exp ran on ACT during our projections)
                    if pending_norm:
                        _emit_norm()

                    # ---------- attention rounds (one s_k tile each) ----------
                    # st [128, 1024] = [h0 scores | h1 scores] for tile t.
                    # bufs=2 double-buffers: scores(t+1) run during exp(t).
                    n_t = 4 * c + 4
                    ot0 = psO.tile([65, 512], f32, tag="ot0")
                    ot1 = psO.tile([65, 512], f32, tag="ot1")
                    wo_work = wo_queue
                    wo_queue = []
                    wo_per_round = (len(wo_work) + n_t - 1) // n_t if wo_work else 0
                    # s_k tiles DESCENDING: the widest PV matmul goes first
                    # (start=True clears the whole bank; later narrower tiles
                    # overwrite-where-unwritten / accumulate-where-written),
                    # making causally-trimmed ragged accumulation legal. The
                    # mask then only covers the aligned 128x128 block.
                    for t in range(n_t - 1, -1, -1):
                        st = psS.tile([128, 1024], f32, tag="st")
                        soff = max(0, (t - 4 * c) * 128) if trim else 0
                        for h in (0, 1):
                            hp = h * 64
                            nc.tensor.matmul(
                                st[:, h * 512 + soff:(h + 1) * 512],
                                kT[hp:hp + 64, t * 128:(t + 1) * 128],
                                qT[hp:hp + 64, c * 512 + soff:(c + 1) * 512],
                                start=True, stop=True)
                        pt = ptp.tile([128, 1024], bf16, tag="pt")
                        nc.scalar.activation(pt[:], st[:], Exp, scale=float(SCALE))
                        off = soff
                        if t >= 4 * c:  # diagonal tile: mask aligned block
                            dd = t - 4 * c
                            mw = 128 if trim else (dd + 1) * 128
                            for h in (0, 1):
                                nc.vector.tensor_tensor(
                                    out=pt[:, h * 512 + off:h * 512 + off + mw],
                                    in0=pt[:, h * 512 + off:h * 512 + off + mw],
                                    in1=masks_sb[:, dd * 512 + off:
                                                 dd * 512 + off + mw],
                                    op=mult)
                        for h, otx in ((0, ot0), (1, ot1)):
                            nc.tensor.matmul(
                                otx[0:65, off:512],
                                v_sb[:, t * VW + h * 65:t * VW + h * 65 + 65],
                                pt[:, h * 512 + off:(h + 1) * 512],
                                start=(t == n_t - 1), stop=(t == 0))
                        for _ in range(wo_per_round):
                            if wo_work:
                                _emit_wo(wo_work.pop(0))
                    while wo_work:
                        _emit_wo(wo_work.pop(0))

                    # ---------- rowsum -> 1/r (ACT only; rest deferred) ----------
                    lnr = smp.tile([1, 1024], f32, tag="lnr")
                    nc.scalar.activation(lnr[0:1, 0:512], ot0[64:65, :], Ln)
                    nc.scalar.activation(lnr[0:1, 512:1024], ot1[64:65, :], Ln)
                    rcp = smp.tile([1, 1024], bf16, tag="rcp")
                    nc.scalar.activation(rcp[:], lnr[:], Exp, scale=-1.0)
                    pending_norm.append((b, c, ot0, ot1, oT, rcp))
            _emit_norm()  # the very last chunk's normalize + WO drain
            while wo_queue:
                _emit_wo(wo_queue.pop(0), tail=True)
    if split_waits:
        _split_waits(nc, mybir)
    return nc


def _split_waits(nc, mybir):
    """This walrus build encodes at most ONE sync wait per instruction.
    Instructions with a single wait keep it inline (free); only multi-
    wait instructions get the extra waits hoisted onto same-engine NoOps
    issued immediately before — semantically identical: the sequencer
    blocks at the NoOp instead."""
    ctr = [0]
    for fn in nc.m.functions:
        for blk in fn.blocks:
            new_insts = []
            for ins in blk.instructions:
                si = getattr(ins, "sync_info", None)
                waits = list(si.on_wait) if si is not None and si.on_wait else []
                if len(waits) > 1:
                    for w in waits[:-1]:
                        ctr[0] += 1
                        nop = mybir.InstNoOp(name=f"WSPLIT-{ctr[0]}", ins=[], outs=[])
                        nop.engine = ins.engine
                        nop.sync_info = mybir.SyncInfo(on_wait=[w], on_update=[])
                        new_insts.append(nop)
                    ins.sync_info = mybir.SyncInfo(
                        on_wait=[waits[-1]], on_update=list(si.on_update or []))
                new_insts.append(ins)
            blk.instructions = new_insts


def _marshal(Q, K, V, WQ, WK, WV, WO):
    Q = np.asarray(Q, dtype=np.float32)
    K = np.asarray(K, dtype=np.float32)
    V = np.asarray(V, dtype=np.float32)
    WQ = np.asarray(WQ, dtype=np.float32)
    WK = np.asarray(WK, dtype=np.float32)
    WV = np.asarray(WV, dtype=np.float32)
    WO = np.asarray(WO, dtype=np.float32)

    import ml_dtypes
    bf = ml_dtypes.bfloat16
    xtq = np.ascontiguousarray(Q.transpose(1, 2, 0)).astype(bf)  # [B, DM, S]
    xtk = np.ascontiguousarray(K.transpose(1, 2, 0)).astype(bf)
    xtv = np.ascontiguousarray(V.transpose(1, 2, 0)).astype(bf)

    masks = np.zeros((4, 128, 512), dtype=bf)
    kk = np.arange(128)[:, None]
    qq = np.arange(512)[None, :]
    for d in range(4):
        masks[d] = (d * 128 + kk <= qq).astype(bf)

    in_maps = []
    for core in range(NCORES):
        h0 = core * HEADS_PER_CORE
        wql = np.ascontiguousarray(np.concatenate([WQ[h0], WQ[h0 + 1]], axis=1)).astype(bf)
        wkl = np.ascontiguousarray(np.concatenate([WK[h0], WK[h0 + 1]], axis=1)).astype(bf)
        wvl = np.ascontiguousarray(np.concatenate([WV[h0], WV[h0 + 1]], axis=1)).astype(bf)
        wol = np.ascontiguousarray(WO[h0 * DV:(h0 + 2) * DV, :]).astype(bf)
        in_maps.append({
            "xtq": xtq, "xtk": xtk, "xtv": xtv,
            "wq": wql, "wk": wkl, "wv": wvl, "wo": wol,
            "masks": masks,
        })
    return in_maps


LAST_RESULTS = None


def kernel(Q, K, V, WQ, WK, WV, WO):
    global LAST_RESULTS
    from concourse.bass_utils import run_bass_kernel_spmd

    if "nc" not in _CACHE:
        _CACHE["nc"] = build_nc()
    nc = _CACHE["nc"]

    in_maps = _marshal(Q, K, V, WQ, WK, WV, WO)
    res = run_bass_kernel_spmd(nc, in_maps, core_ids=list(range(NCORES)))
    LAST_RESULTS = res
    out = np.zeros((S, B, DM), dtype=np.float32)
    for r in res.results:
        out += np.asarray(r["y"]).astype(np.float32)
    return out
